# revision 3
# baseline (speedup 1.0000x reference)
"""BoundaryMaxPooling Trainium2 kernel.

Reference computation (B=16, C2=512, T=Tf=126):
  - segment windows [s0,s1) / [e0,e1) derived from segments[0] only (batch-0 row)
  - out[b, c, t]      = max_{j in [s0(t), s1(t))} feature[b, c, j]       (c < 256)
  - out[b, 256+c, t]  = max_{j in [e0(t), e1(t))} feature[b, 256+c, j]

Device algorithm (per core, 2 batches, data-parallel over batch):
  Sparse-table (log-level) range max with j on SBUF partitions:
    L_0[j, c'] = feature^T   (c' = half*512 + b*256 + c, 1024 columns)
    L_{k+1}[j] = max(L_k[j], L_k[j + 2^k])   for j in [0, 127 - 2^{k+1})
  The partition shift L_k[j + 2^k] is produced by the TensorEngine with an
  exact one-hot band matrix (compute engines cannot read SBUF at partition
  offsets other than 0/32/64/96; DMA/PE can).  Window max for window length
  L, k = floor(log2 L):
    out[t] = max(L_k[a(t)], L_k[b(t)]),  a = lo, b = hi - 2^k
  Both lookups are exact one-hot gather matmuls (float32r, full PE rate)
  accumulated over levels in PSUM; a zero one-hot column contributes exact 0.
  Host precomputes all index matrices from segments[0] (they are replicated
  across cores), pre-transposes features per core, and reassembles/transposes
  the output; empty end-windows (e0 == -1) are data-independent and set to
  float32 min on the host, matching the reference.
"""

import os
import sys

import numpy as np

if os.path.isdir("/opt/trn_rl_repo") and "/opt/trn_rl_repo" not in sys.path:
    sys.path.insert(0, "/opt/trn_rl_repo")

import concourse.bass as bass  # noqa: E402
from concourse import bacc, mybir, tile  # noqa: E402
from concourse.bass_utils import run_bass_kernel_spmd  # noqa: E402

B, C2, T = 16, 512, 126
C = C2 // 2  # 256
NCORES = 8
BPC = B // NCORES  # batches per core = 2
CPRIME = BPC * C2  # 1024 columns per core
NLEV = 7
KS = [127 - (1 << k) for k in range(NLEV)]  # valid rows of level k

F32 = mybir.dt.float32
F32R = mybir.dt.float32r
MAX = mybir.AluOpType.max

_CACHE = {}

# test.py hooks: set TRACE=True before calling kernel() to capture a profile.
TRACE = False
LAST_RESULTS = None


def _build_module():
    nc = bacc.Bacc(None, target_bir_lowering=False, debug=False)

    ft = nc.dram_tensor("ft", [T, CPRIME], F32R, kind="ExternalInput")
    g_dram = {}
    for gi in range(2):
        for h in range(2):
            for k in range(NLEV):
                g_dram[(gi, h, k)] = nc.dram_tensor(
                    f"g_{gi}_{h}_{k}", [KS[k], T], F32R, kind="ExternalInput"
                )
    sh_dram = {
        k: nc.dram_tensor(f"sh_{k}", [KS[k], KS[k + 1]], F32R, kind="ExternalInput")
        for k in range(NLEV - 1)
    }
    out = nc.dram_tensor("out", [T, CPRIME], F32, kind="ExternalOutput")

    with tile.TileContext(nc) as tc:
        with (
            tc.tile_pool(name="lv", bufs=1) as lvp,
            tc.tile_pool(name="gw", bufs=1) as gwp,
            tc.tile_pool(name="acc", bufs=1, space=bass.MemorySpace.PSUM) as accp,
            tc.tile_pool(name="shp", bufs=2, space=bass.MemorySpace.PSUM) as shpp,
        ):
            L = [lvp.tile([KS[k], CPRIME], F32R, name=f"L{k}") for k in range(NLEV)]
            nc.sync.dma_start(out=L[0][:, :], in_=ft[:, :])

            sh_t = {}
            for k in range(NLEV - 1):
                t_ = gwp.tile([KS[k], KS[k + 1]], F32R, name=f"sht{k}")
                nc.sync.dma_start(out=t_[:, :], in_=sh_dram[k][:, :])
                sh_t[k] = t_
            g_t = {}
            for k in range(NLEV):
                for gi in range(2):
                    for h in range(2):
                        t_ = gwp.tile([KS[k], T], F32R, name=f"gt{gi}{h}{k}")
                        nc.sync.dma_start(out=t_[:, :], in_=g_dram[(gi, h, k)][:, :])
                        g_t[(gi, h, k)] = t_

            p_acc = [accp.tile([T, CPRIME], F32, name=f"pacc{gi}") for gi in range(2)]

            for k in range(NLEV):
                # shift for next level first: it is on the critical path
                if k < NLEV - 1:
                    shp = shpp.tile([KS[k + 1], CPRIME], F32, name=f"shp{k}", tag="shp")
                    for half in range(2):
                        nc.tensor.matmul(
                            shp[:, half * 512 : (half + 1) * 512],
                            sh_t[k][:, :],
                            L[k][:, half * 512 : (half + 1) * 512],
                            start=True,
                            stop=True,
                        )
                # gathers from this level (accumulate into p_acc)
                for gi in range(2):
                    for h in range(2):
                        nc.tensor.matmul(
                            p_acc[gi][:, h * 512 : (h + 1) * 512],
                            g_t[(gi, h, k)][:, :],
                            L[k][:, h * 512 : (h + 1) * 512],
                            start=(k == 0),
                            stop=(k == NLEV - 1),
                        )
                if k < NLEV - 1:
                    nc.vector.tensor_max(
                        L[k + 1][:, :],
                        L[k][0 : KS[k + 1], :],
                        shp[:, :],
                    )

            s1t = gwp.tile([T, CPRIME], F32, name="s1t")
            nc.scalar.copy(out=s1t[:, :], in_=p_acc[0][:, :])
            ot = gwp.tile([T, CPRIME], F32, name="ot")
            nc.vector.tensor_max(ot[:, :], s1t[:, :], p_acc[1][:, :])
            nc.sync.dma_start(out=out[:, :], in_=ot[:, :])

    nc.compile()
    return nc


def _host_windows(segments):
    """Replicates the reference's index math on segments[0]. Returns per half
    (lo, hi) clamped windows plus the empty mask."""
    seg = np.clip(segments.astype(np.float32), 0.0, 125.0)
    row = seg[0]  # [T, 4]
    s0 = np.floor(row[:, 0]).astype(np.int32)
    s1 = np.ceil(row[:, 1]).astype(np.int32)
    s1 = np.where(s0 == s1, s1 + 1, s1)
    e0 = np.floor(row[:, 2]).astype(np.int32)
    e1 = np.ceil(row[:, 3]).astype(np.int32)
    e0 = np.where(e0 == e1, e0 - 1, e0)

    halves = []
    for lo, hi in ((s0, s1), (e0, e1)):
        lo_c = np.maximum(lo, 0)
        hi_c = np.minimum(hi, T)
        empty = lo_c >= hi_c
        halves.append((lo_c, hi_c, empty))
    return halves


def _host_matrices(segments):
    halves = _host_windows(segments)
    g = {
        (gi, h, k): np.zeros((KS[k], T), np.float32)
        for gi in range(2)
        for h in range(2)
        for k in range(NLEV)
    }
    for h, (lo, hi, empty) in enumerate(halves):
        for t in range(T):
            if empty[t]:
                continue
            ln = int(hi[t] - lo[t])
            k = ln.bit_length() - 1
            a = int(lo[t])
            b = int(hi[t]) - (1 << k)
            g[(0, h, k)][a, t] = 1.0
            g[(1, h, k)][b, t] = 1.0
    sh = {}
    for k in range(NLEV - 1):
        m = np.zeros((KS[k], KS[k + 1]), np.float32)
        s = 1 << k
        for j in range(KS[k + 1]):
            m[j + s, j] = 1.0
        sh[k] = m
    return g, sh, halves


def _tf32_round(x):
    """Round float32 to tf32 (10 explicit mantissa bits), round-nearest-even."""
    b = x.view(np.uint32)
    keep = np.uint32(0xFFFFE000)
    round_bit = ((b >> np.uint32(13)) & np.uint32(1)) + np.uint32(0x0FFF)
    b = (b + round_bit) & keep
    return b.view(np.float32)


def _shard_feature(feature):
    """Core i gets batches [2i, 2i+2) as [T, CPRIME] with
    c' = half*512 + local_batch*256 + channel_within_half."""
    fts = []
    for i in range(NCORES):
        pair = _tf32_round(np.ascontiguousarray(feature[BPC * i : BPC * (i + 1)]))
        arr = pair.reshape(BPC, 2, C, T)  # [b, h, c, j]
        arr = np.ascontiguousarray(arr.transpose(3, 1, 0, 2).reshape(T, CPRIME))
        fts.append(arr)
    return fts


def _unshard(results, halves):
    out = np.empty((B, C2, T), np.float32)
    for i in range(NCORES):
        r = np.asarray(results[i]["out"], dtype=np.float32)  # [T, CPRIME]
        arr = r.reshape(T, 2, BPC, C).transpose(2, 1, 3, 0)  # [b, h, c, t]
        out[BPC * i : BPC * (i + 1)] = arr.reshape(BPC, C2, T)
    neg = np.finfo(np.float32).min
    for h, (_, _, empty) in enumerate(halves):
        if empty.any():
            out[:, h * C : (h + 1) * C, empty] = neg
    return out


def kernel(feature, segments):
    global LAST_RESULTS
    feature = np.ascontiguousarray(feature, dtype=np.float32)
    segments = np.ascontiguousarray(segments, dtype=np.float32)

    if "nc" not in _CACHE:
        _CACHE["nc"] = _build_module()
    nc = _CACHE["nc"]

    g, sh, halves = _host_matrices(segments)
    fts = _shard_feature(feature)

    shared = {f"g_{gi}_{h}_{k}": g[(gi, h, k)] for (gi, h, k) in g}
    shared.update({f"sh_{k}": sh[k] for k in sh})
    in_maps = [{"ft": fts[i], **shared} for i in range(NCORES)]

    res = run_bass_kernel_spmd(nc, in_maps, list(range(NCORES)), trace=TRACE)
    LAST_RESULTS = res
    return _unshard(res.results, halves)


# revision 4
# speedup vs baseline: 1.1082x; 1.1082x over previous
"""BoundaryMaxPooling Trainium2 kernel.

Reference computation (B=16, C2=512, T=Tf=126):
  - segment windows [s0,s1) / [e0,e1) derived from segments[0] only (batch-0 row)
  - out[b, c, t]      = max_{j in [s0(t), s1(t))} feature[b, c, j]       (c < 256)
  - out[b, 256+c, t]  = max_{j in [e0(t), e1(t))} feature[b, 256+c, j]

Device algorithm (per core, 2 batches, data-parallel over batch):
  Sparse-table (log-level) range max with j on SBUF partitions:
    L_0[j, c'] = feature^T   (c' = half*512 + b*256 + c, 1024 columns)
    L_{k+1}[j] = max(L_k[j], L_k[j + 2^k])   for j in [0, 127 - 2^{k+1})
  The partition shift L_k[j + 2^k] is produced by the TensorEngine with an
  exact one-hot band matrix (compute engines cannot read SBUF at partition
  offsets other than 0/32/64/96; DMA/PE can).  Window max for window length
  L, k = floor(log2 L):
    out[t] = max(L_k[a(t)], L_k[b(t)]),  a = lo, b = hi - 2^k
  Both lookups are exact one-hot gather matmuls (float32r, full PE rate)
  accumulated over levels in PSUM; a zero one-hot column contributes exact 0.
  Host precomputes all index matrices from segments[0] (they are replicated
  across cores), pre-transposes features per core, and reassembles/transposes
  the output; empty end-windows (e0 == -1) are data-independent and set to
  float32 min on the host, matching the reference.
"""

import os
import sys

import numpy as np

if os.path.isdir("/opt/trn_rl_repo") and "/opt/trn_rl_repo" not in sys.path:
    sys.path.insert(0, "/opt/trn_rl_repo")

import concourse.bass as bass  # noqa: E402
from concourse import bacc, mybir, tile  # noqa: E402
from concourse.bass_utils import run_bass_kernel_spmd  # noqa: E402

B, C2, T = 16, 512, 126
C = C2 // 2  # 256
NCORES = 8
BPC = B // NCORES  # batches per core = 2
CPRIME = BPC * C2  # 1024 columns per core
NLEV = 7
KS = [127 - (1 << k) for k in range(NLEV)]  # valid rows of level k

F32 = mybir.dt.float32
F32R = mybir.dt.float32r
MAX = mybir.AluOpType.max

_CACHE = {}

# test.py hooks: set TRACE=True before calling kernel() to capture a profile.
TRACE = False
LAST_RESULTS = None


def _wts_layout():
    """Column layout of the flat weight tensor [T, total]: level-major so the
    k=0 group (needed first) is a contiguous prefix."""
    offs = {}
    off = 0
    for k in range(NLEV):
        if k < NLEV - 1:
            offs[("sh", k)] = (off, KS[k + 1])
            off += KS[k + 1]
        for gi in range(2):
            for h in range(2):
                offs[("g", gi, h, k)] = (off, T)
                off += T
    return offs, off


def _build_module():
    nc = bacc.Bacc(None, target_bir_lowering=False, debug=False)

    ft = nc.dram_tensor("ft", [T, CPRIME], F32R, kind="ExternalInput")
    offs, total = _wts_layout()
    wts = nc.dram_tensor("wts", [T, total], F32R, kind="ExternalInput")
    out = nc.dram_tensor("out", [T, CPRIME], F32, kind="ExternalOutput")

    with tile.TileContext(nc) as tc:
        with (
            tc.tile_pool(name="lv", bufs=1) as lvp,
            tc.tile_pool(name="gw", bufs=1) as gwp,
            tc.tile_pool(name="acc", bufs=1, space=bass.MemorySpace.PSUM) as accp,
            tc.tile_pool(name="shp", bufs=2, space=bass.MemorySpace.PSUM) as shpp,
        ):
            L = [lvp.tile([KS[k], CPRIME], F32R, name=f"L{k}") for k in range(NLEV)]
            nc.sync.dma_start(out=L[0][:, :], in_=ft[:, :])

            wt = gwp.tile([T, total], F32R, name="wt")
            lev0_end = offs[("g", 1, 1, 0)][0] + T  # end of the k=0 group
            nc.sync.dma_start(out=wt[:, 0:lev0_end], in_=wts[:, 0:lev0_end])
            nc.sync.dma_start(out=wt[:, lev0_end:total], in_=wts[:, lev0_end:total])

            def sh_ap(k):
                o, n = offs[("sh", k)]
                return wt[0 : KS[k], o : o + n]

            def g_ap(gi, h, k):
                o, n = offs[("g", gi, h, k)]
                return wt[0 : KS[k], o : o + n]

            p_acc = [accp.tile([T, CPRIME], F32, name=f"pacc{gi}") for gi in range(2)]

            for k in range(NLEV):
                # shift for next level first: it is on the critical path
                if k < NLEV - 1:
                    shp = shpp.tile([KS[k + 1], CPRIME], F32, name=f"shp{k}", tag="shp")
                    for half in range(2):
                        nc.tensor.matmul(
                            shp[:, half * 512 : (half + 1) * 512],
                            sh_ap(k),
                            L[k][:, half * 512 : (half + 1) * 512],
                            start=True,
                            stop=True,
                        )
                # gathers from this level (accumulate into p_acc)
                for gi in range(2):
                    for h in range(2):
                        nc.tensor.matmul(
                            p_acc[gi][:, h * 512 : (h + 1) * 512],
                            g_ap(gi, h, k),
                            L[k][:, h * 512 : (h + 1) * 512],
                            start=(k == 0),
                            stop=(k == NLEV - 1),
                        )
                if k < NLEV - 1:
                    nc.vector.tensor_max(
                        L[k + 1][:, :],
                        L[k][0 : KS[k + 1], :],
                        shp[:, :],
                    )

            s1t = gwp.tile([T, CPRIME], F32, name="s1t")
            nc.scalar.copy(out=s1t[:, :], in_=p_acc[0][:, :])
            ot = gwp.tile([T, CPRIME], F32, name="ot")
            nc.vector.tensor_max(ot[:, :], s1t[:, :], p_acc[1][:, :])
            nc.sync.dma_start(out=out[:, :], in_=ot[:, :])

    nc.compile()
    return nc


def _host_windows(segments):
    """Replicates the reference's index math on segments[0]. Returns per half
    (lo, hi) clamped windows plus the empty mask."""
    seg = np.clip(segments.astype(np.float32), 0.0, 125.0)
    row = seg[0]  # [T, 4]
    s0 = np.floor(row[:, 0]).astype(np.int32)
    s1 = np.ceil(row[:, 1]).astype(np.int32)
    s1 = np.where(s0 == s1, s1 + 1, s1)
    e0 = np.floor(row[:, 2]).astype(np.int32)
    e1 = np.ceil(row[:, 3]).astype(np.int32)
    e0 = np.where(e0 == e1, e0 - 1, e0)

    halves = []
    for lo, hi in ((s0, s1), (e0, e1)):
        lo_c = np.maximum(lo, 0)
        hi_c = np.minimum(hi, T)
        empty = lo_c >= hi_c
        halves.append((lo_c, hi_c, empty))
    return halves


def _host_matrices(segments):
    halves = _host_windows(segments)
    g = {
        (gi, h, k): np.zeros((KS[k], T), np.float32)
        for gi in range(2)
        for h in range(2)
        for k in range(NLEV)
    }
    for h, (lo, hi, empty) in enumerate(halves):
        for t in range(T):
            if empty[t]:
                continue
            ln = int(hi[t] - lo[t])
            k = ln.bit_length() - 1
            a = int(lo[t])
            b = int(hi[t]) - (1 << k)
            g[(0, h, k)][a, t] = 1.0
            g[(1, h, k)][b, t] = 1.0
    sh = {}
    for k in range(NLEV - 1):
        m = np.zeros((KS[k], KS[k + 1]), np.float32)
        s = 1 << k
        for j in range(KS[k + 1]):
            m[j + s, j] = 1.0
        sh[k] = m
    return g, sh, halves


def _tf32_round(x):
    """Round float32 to tf32 (10 explicit mantissa bits), round-nearest-even."""
    b = x.view(np.uint32)
    keep = np.uint32(0xFFFFE000)
    round_bit = ((b >> np.uint32(13)) & np.uint32(1)) + np.uint32(0x0FFF)
    b = (b + round_bit) & keep
    return b.view(np.float32)


def _shard_feature(feature):
    """Core i gets batches [2i, 2i+2) as [T, CPRIME] with
    c' = half*512 + local_batch*256 + channel_within_half."""
    fts = []
    for i in range(NCORES):
        pair = _tf32_round(np.ascontiguousarray(feature[BPC * i : BPC * (i + 1)]))
        arr = pair.reshape(BPC, 2, C, T)  # [b, h, c, j]
        arr = np.ascontiguousarray(arr.transpose(3, 1, 0, 2).reshape(T, CPRIME))
        fts.append(arr)
    return fts


def _unshard(results, halves):
    out = np.empty((B, C2, T), np.float32)
    for i in range(NCORES):
        r = np.asarray(results[i]["out"], dtype=np.float32)  # [T, CPRIME]
        arr = r.reshape(T, 2, BPC, C).transpose(2, 1, 3, 0)  # [b, h, c, t]
        out[BPC * i : BPC * (i + 1)] = arr.reshape(BPC, C2, T)
    neg = np.finfo(np.float32).min
    for h, (_, _, empty) in enumerate(halves):
        if empty.any():
            out[:, h * C : (h + 1) * C, empty] = neg
    return out


def kernel(feature, segments):
    global LAST_RESULTS
    feature = np.ascontiguousarray(feature, dtype=np.float32)
    segments = np.ascontiguousarray(segments, dtype=np.float32)

    if "nc" not in _CACHE:
        _CACHE["nc"] = _build_module()
    nc = _CACHE["nc"]

    g, sh, halves = _host_matrices(segments)
    fts = _shard_feature(feature)

    offs, total = _wts_layout()
    wts = np.zeros((T, total), np.float32)
    for k in range(NLEV):
        if k < NLEV - 1:
            o, n = offs[("sh", k)]
            wts[: KS[k], o : o + n] = sh[k]
        for gi in range(2):
            for h in range(2):
                o, n = offs[("g", gi, h, k)]
                wts[: KS[k], o : o + n] = g[(gi, h, k)]
    in_maps = [{"ft": fts[i], "wts": wts} for i in range(NCORES)]

    res = run_bass_kernel_spmd(nc, in_maps, list(range(NCORES)), trace=TRACE)
    LAST_RESULTS = res
    return _unshard(res.results, halves)


# revision 5
# speedup vs baseline: 1.2543x; 1.1318x over previous
"""BoundaryMaxPooling Trainium2 kernel.

Reference computation (B=16, C2=512, T=Tf=126):
  - segment windows [s0,s1) / [e0,e1) derived from segments[0] only (batch-0 row)
  - out[b, c, t]      = max_{j in [s0(t), s1(t))} feature[b, c, j]       (c < 256)
  - out[b, 256+c, t]  = max_{j in [e0(t), e1(t))} feature[b, 256+c, j]

Device algorithm (per core, 2 batches, data-parallel over batch):
  Sparse-table (log-level) range max with j on SBUF partitions:
    L_0[j, c'] = feature^T   (c' = half*512 + b*256 + c, 1024 columns)
    L_{k+1}[j] = max(L_k[j], L_k[j + 2^k])   for j in [0, 127 - 2^{k+1})
  The partition shift L_k[j + 2^k] is produced by the TensorEngine with an
  exact one-hot band matrix (compute engines cannot read SBUF at partition
  offsets other than 0/32/64/96; DMA/PE can).  Window max for window length
  L, k = floor(log2 L):
    out[t] = max(L_k[a(t)], L_k[b(t)]),  a = lo, b = hi - 2^k
  Both lookups are exact one-hot gather matmuls (float32r, full PE rate)
  accumulated over levels in PSUM; a zero one-hot column contributes exact 0.
  Host precomputes all index matrices from segments[0] (they are replicated
  across cores), pre-transposes features per core, and reassembles/transposes
  the output; empty end-windows (e0 == -1) are data-independent and set to
  float32 min on the host, matching the reference.
"""

import os
import sys

import numpy as np

if os.path.isdir("/opt/trn_rl_repo") and "/opt/trn_rl_repo" not in sys.path:
    sys.path.insert(0, "/opt/trn_rl_repo")

import concourse.bass as bass  # noqa: E402
from concourse import bacc, mybir, tile  # noqa: E402
from concourse.bass_utils import run_bass_kernel_spmd  # noqa: E402

B, C2, T = 16, 512, 126
C = C2 // 2  # 256
NCORES = 8
BPC = B // NCORES  # batches per core = 2
CPRIME = BPC * C2  # 1024 columns per core
NLEV = 7
KS = [127 - (1 << k) for k in range(NLEV)]  # valid rows of level k

F32 = mybir.dt.float32
F32R = mybir.dt.float32r
MAX = mybir.AluOpType.max

_CACHE = {}

# test.py hooks: set TRACE=True before calling kernel() to capture a profile.
TRACE = False
LAST_RESULTS = None


def _wts_layout():
    """Column layout of the flat weight tensor [T, total]: level-major so the
    k=0 group (needed first) is a contiguous prefix."""
    offs = {}
    off = 0
    for k in range(NLEV):
        if k < NLEV - 1:
            offs[("sh", k)] = (off, KS[k + 1])
            off += KS[k + 1]
        for gi in range(2):
            for h in range(2):
                offs[("g", gi, h, k)] = (off, T)
                off += T
    return offs, off


def _build_module():
    nc = bacc.Bacc(None, target_bir_lowering=False, debug=False)

    ft = nc.dram_tensor("ft", [T, CPRIME], F32R, kind="ExternalInput")
    offs, total = _wts_layout()
    wts = nc.dram_tensor("wts", [T, total], F32R, kind="ExternalInput")
    out = nc.dram_tensor("out", [T, CPRIME], F32, kind="ExternalOutput")

    with tile.TileContext(nc) as tc:
        with (
            tc.tile_pool(name="lv", bufs=1) as lvp,
            tc.tile_pool(name="gw", bufs=1) as gwp,
            tc.tile_pool(name="acc", bufs=1, space=bass.MemorySpace.PSUM) as accp,
            tc.tile_pool(name="shp", bufs=2, space=bass.MemorySpace.PSUM) as shpp,
        ):
            L = [lvp.tile([KS[k], CPRIME], F32R, name=f"L{k}") for k in range(NLEV)]
            nc.sync.dma_start(out=L[0][:, :], in_=ft[:, :])

            wt = gwp.tile([T, total], F32R, name="wt")
            # per-level weight chunks, alternating the two HWDGE queues
            # (SP via nc.sync, ACT via nc.scalar) so descriptor latency
            # overlaps; earlier levels are needed first.
            bounds = []
            for k in range(NLEV):
                lo = offs[("sh", k)][0] if k < NLEV - 1 else offs[("g", 0, 0, k)][0]
                hi = offs[("g", 1, 1, k)][0] + T
                bounds.append((lo, hi))
            for k, (lo, hi) in enumerate(bounds):
                eng = nc.scalar if k % 2 == 0 else nc.sync
                eng.dma_start(out=wt[:, lo:hi], in_=wts[:, lo:hi])

            def sh_ap(k):
                o, n = offs[("sh", k)]
                return wt[0 : KS[k], o : o + n]

            def g_ap(gi, h, k):
                o, n = offs[("g", gi, h, k)]
                return wt[0 : KS[k], o : o + n]

            p_acc = [accp.tile([T, CPRIME], F32, name=f"pacc{gi}") for gi in range(2)]

            for k in range(NLEV):
                # shift for next level first: it is on the critical path
                if k < NLEV - 1:
                    shp = shpp.tile([KS[k + 1], CPRIME], F32, name=f"shp{k}", tag="shp")
                    for half in range(2):
                        nc.tensor.matmul(
                            shp[:, half * 512 : (half + 1) * 512],
                            sh_ap(k),
                            L[k][:, half * 512 : (half + 1) * 512],
                            start=True,
                            stop=True,
                        )
                # gathers from this level (accumulate into p_acc)
                for gi in range(2):
                    for h in range(2):
                        nc.tensor.matmul(
                            p_acc[gi][:, h * 512 : (h + 1) * 512],
                            g_ap(gi, h, k),
                            L[k][:, h * 512 : (h + 1) * 512],
                            start=(k == 0),
                            stop=(k == NLEV - 1),
                        )
                if k < NLEV - 1:
                    nc.vector.tensor_max(
                        L[k + 1][:, :],
                        L[k][0 : KS[k + 1], :],
                        shp[:, :],
                    )

            s1t = gwp.tile([T, CPRIME], F32, name="s1t")
            nc.scalar.copy(out=s1t[:, :], in_=p_acc[0][:, :])
            ot = gwp.tile([T, CPRIME], F32, name="ot")
            nc.vector.tensor_max(ot[:, :], s1t[:, :], p_acc[1][:, :])
            nc.sync.dma_start(out=out[:, :], in_=ot[:, :])

    nc.compile()
    return nc


def _host_windows(segments):
    """Replicates the reference's index math on segments[0]. Returns per half
    (lo, hi) clamped windows plus the empty mask."""
    seg = np.clip(segments.astype(np.float32), 0.0, 125.0)
    row = seg[0]  # [T, 4]
    s0 = np.floor(row[:, 0]).astype(np.int32)
    s1 = np.ceil(row[:, 1]).astype(np.int32)
    s1 = np.where(s0 == s1, s1 + 1, s1)
    e0 = np.floor(row[:, 2]).astype(np.int32)
    e1 = np.ceil(row[:, 3]).astype(np.int32)
    e0 = np.where(e0 == e1, e0 - 1, e0)

    halves = []
    for lo, hi in ((s0, s1), (e0, e1)):
        lo_c = np.maximum(lo, 0)
        hi_c = np.minimum(hi, T)
        empty = lo_c >= hi_c
        halves.append((lo_c, hi_c, empty))
    return halves


def _host_matrices(segments):
    halves = _host_windows(segments)
    g = {
        (gi, h, k): np.zeros((KS[k], T), np.float32)
        for gi in range(2)
        for h in range(2)
        for k in range(NLEV)
    }
    for h, (lo, hi, empty) in enumerate(halves):
        for t in range(T):
            if empty[t]:
                continue
            ln = int(hi[t] - lo[t])
            k = ln.bit_length() - 1
            a = int(lo[t])
            b = int(hi[t]) - (1 << k)
            g[(0, h, k)][a, t] = 1.0
            g[(1, h, k)][b, t] = 1.0
    sh = {}
    for k in range(NLEV - 1):
        m = np.zeros((KS[k], KS[k + 1]), np.float32)
        s = 1 << k
        for j in range(KS[k + 1]):
            m[j + s, j] = 1.0
        sh[k] = m
    return g, sh, halves


def _tf32_round(x):
    """Round float32 to tf32 (10 explicit mantissa bits), round-nearest-even."""
    b = x.view(np.uint32)
    keep = np.uint32(0xFFFFE000)
    round_bit = ((b >> np.uint32(13)) & np.uint32(1)) + np.uint32(0x0FFF)
    b = (b + round_bit) & keep
    return b.view(np.float32)


def _shard_feature(feature):
    """Core i gets batches [2i, 2i+2) as [T, CPRIME] with
    c' = half*512 + local_batch*256 + channel_within_half."""
    fts = []
    for i in range(NCORES):
        pair = _tf32_round(np.ascontiguousarray(feature[BPC * i : BPC * (i + 1)]))
        arr = pair.reshape(BPC, 2, C, T)  # [b, h, c, j]
        arr = np.ascontiguousarray(arr.transpose(3, 1, 0, 2).reshape(T, CPRIME))
        fts.append(arr)
    return fts


def _unshard(results, halves):
    out = np.empty((B, C2, T), np.float32)
    for i in range(NCORES):
        r = np.asarray(results[i]["out"], dtype=np.float32)  # [T, CPRIME]
        arr = r.reshape(T, 2, BPC, C).transpose(2, 1, 3, 0)  # [b, h, c, t]
        out[BPC * i : BPC * (i + 1)] = arr.reshape(BPC, C2, T)
    neg = np.finfo(np.float32).min
    for h, (_, _, empty) in enumerate(halves):
        if empty.any():
            out[:, h * C : (h + 1) * C, empty] = neg
    return out


def kernel(feature, segments):
    global LAST_RESULTS
    feature = np.ascontiguousarray(feature, dtype=np.float32)
    segments = np.ascontiguousarray(segments, dtype=np.float32)

    if "nc" not in _CACHE:
        _CACHE["nc"] = _build_module()
    nc = _CACHE["nc"]

    g, sh, halves = _host_matrices(segments)
    fts = _shard_feature(feature)

    offs, total = _wts_layout()
    wts = np.zeros((T, total), np.float32)
    for k in range(NLEV):
        if k < NLEV - 1:
            o, n = offs[("sh", k)]
            wts[: KS[k], o : o + n] = sh[k]
        for gi in range(2):
            for h in range(2):
                o, n = offs[("g", gi, h, k)]
                wts[: KS[k], o : o + n] = g[(gi, h, k)]
    in_maps = [{"ft": fts[i], "wts": wts} for i in range(NCORES)]

    res = run_bass_kernel_spmd(nc, in_maps, list(range(NCORES)), trace=TRACE)
    LAST_RESULTS = res
    return _unshard(res.results, halves)
